# revision 25
# baseline (speedup 1.0000x reference)
"""Trainium2 Bass kernel for nn_AlphaModel (3DGS EWA conic rasterization term).

Math: the reference output inside[b, p] is a quadratic polynomial in the pixel
coordinates (tx, ty) with per-camera coefficients:

    inside[b,p] = a_yy[b]*ty^2 + a_xx[b]*tx^2 + a_xy[b]*tx*ty
                + a_y[b]*ty + a_x[b]*tx + a_0[b]

so the [B, P] output is a rank-6 contraction  coef[B,6] @ basis[6,P].
Per-camera coefficients are derived on-device from x (the sharded input);
basis rows are derived on-device from tile_coord.  Data-parallel over B
across 8 cores; no cross-device communication.

Derivation (with q00/q11/q01 the FX/FY-factored 2D-covariance entries,
wx = vx/FX, wy = vy/FY, D = q00*q11 - q01^2, u = zc^2):
    a_yy = q00 u^2/(FY^2 D)        a_xx = q11 u^2/(FX^2 D)
    a_xy = -2 q01 u^2/(FX FY D)
    a_y  = -2u (q00 wy - q01 wx)/(FY D)
    a_x  = -2u (q11 wx - q01 wy)/(FX D)
    a_0  = (q11 wx^2 + q00 wy^2 - 2 q01 wx wy)/D
Basis row order: ty^2, tx^2, tx*ty, ty, tx, 1.
"""

import os

import numpy as np

B = 2048
P = 16384
NCORES = 8
BLOC = B // NCORES          # 256 cameras per core
NBLK = BLOC // 128          # 2 partition blocks per core
NCHUNK = P // 512           # 32 pixel chunks of 512
FX = 2343.0242837919386
FY = 2343.0242837919386
CX = 2560 / 2.0
CY = 1440 / 2.0

# matmul dtype strategy:
#   "f32"  - exact fp32 PE matmul (4 cyc/row, slow)
#   "f32r" - single-pass reduced-precision fp32 (1 cyc/row, ~7e-4 rel err)
#   "hilo" - bf16 hi/lo split, K=18 (1 cyc/row, ~1e-5 rel err)
MM_MODE = os.environ.get("ALPHA_MM_MODE", "hilo")

_cached = {}


def _build(mm_mode: str):
    import concourse.bacc as bacc
    import concourse.mybir as mybir
    import concourse.tile as tile
    from concourse import masks

    f32 = mybir.dt.float32
    bf16 = mybir.dt.bfloat16
    mmdt = {"f32": f32, "f32r": mybir.dt.float32r, "hilo": bf16}[mm_mode]
    KROWS = 18 if mm_mode == "hilo" else 6
    AX = mybir.AxisListType
    OP = mybir.AluOpType

    nc = bacc.Bacc("TRN2", target_bir_lowering=False, debug=False)

    xc = nc.dram_tensor("xc", [128, 56], f32, kind="ExternalInput")
    bs = nc.dram_tensor("bs", [KROWS, P], mmdt, kind="ExternalInput")
    out = nc.dram_tensor("out", [NBLK, 128, P], f32, kind="ExternalOutput")

    with tile.TileContext(nc) as tc:
        with (
            tc.tile_pool(name="const", bufs=1) as cpool,
            tc.tile_pool(name="obuf", bufs=4) as opool,
            tc.tile_pool(name="psum", bufs=6, space="PSUM") as ppool,
            tc.tile_pool(name="psumT", bufs=1, space="PSUM") as ptpool,
        ):
            # ------- input loads: coef-chain inputs first, own rings -----
            xcb = cpool.tile([128, 56], f32)
            nc.sync.dma_start(xcb[:], xc.ap())
            basis = cpool.tile([KROWS, P], mmdt)
            BW = P // 8
            for k in range(8):
                deng = nc.scalar if k % 2 == 0 else nc.sync
                deng.dma_start(
                    basis[:, k * BW : (k + 1) * BW],
                    bs.ap()[:, k * BW : (k + 1) * BW],
                )

            mb = xcb[:, 32:36]
            ab = xcb[:, 36:45]
            sc3 = xcb[:, 45:48]   # 1/FY^2, 1/FX^2, -2/(FX FY)
            sc2 = xcb[:, 48:50]   # -2/FY, -2/FX

            # ---------------- per-camera coefficients --------------------
            _tn = [0]

            def t(*shape):
                _tn[0] += 1
                return cpool.tile([128, *shape], f32, name=f"sc{_tn[0]}")

            mul, add, sub = (
                nc.vector.tensor_mul,
                nc.vector.tensor_add,
                nc.vector.tensor_sub,
            )

            def stt(o, i0, s, i1, op0, op1):
                nc.vector.scalar_tensor_tensor(o, i0, float(s), i1, op0=op0, op1=op1)

            # means_cam[b, i] = sum_j x[b, i, j] * m[j]   (both blocks at once)
            xv = xcb[:, 0:32].rearrange("p (b i j) -> p b i j", b=NBLK, i=4)[
                :, :, 0:3, :
            ]  # [128, b, i, j=4]
            tmp_mc = t(NBLK, 3, 4)
            mul(
                tmp_mc[:], xv,
                mb.unsqueeze(1).unsqueeze(1).broadcast_to((128, NBLK, 3, 4)),
            )
            mc = t(NBLK, 3)
            nc.vector.reduce_sum(mc[:], tmp_mc[:], axis=AX.X)

            # cov_cam = R A R^T per block (ISA limit: 3 free dims -> per blk)
            cov = t(NBLK, 3, 3)
            av = ab.rearrange("p (j l) -> p l j", l=3)  # [128, l, j]
            for blk in range(NBLK):
                rvb = xcb[:, blk * 16 : blk * 16 + 12].rearrange(
                    "p (i j) -> p i j", j=4
                )[:, :, 0:3]  # [128, i, j]
                tmpT = t(3, 3, 3)  # [128, i, l, j]
                mul(
                    tmpT[:],
                    rvb.unsqueeze(2).broadcast_to((128, 3, 3, 3)),
                    av.unsqueeze(1).broadcast_to((128, 3, 3, 3)),
                )
                TT = t(3, 3)  # [128, i, l]
                nc.vector.reduce_sum(TT[:], tmpT[:], axis=AX.X)
                tmpC = t(3, 3, 3)  # [128, i, k, l]
                mul(
                    tmpC[:],
                    TT[:].unsqueeze(2).broadcast_to((128, 3, 3, 3)),
                    rvb.unsqueeze(1).broadcast_to((128, 3, 3, 3)),
                )
                nc.vector.reduce_sum(cov[:, blk, :, :], tmpC[:], axis=AX.X)

            covf = cov[:].rearrange("p b i k -> p b (i k)")

            # mm9[b,i,j] = mc_i * mc_j  (outer product of means)
            mm9 = t(NBLK, 3, 3)
            mul(
                mm9[:],
                mc[:].unsqueeze(3).broadcast_to((128, NBLK, 3, 3)),
                mc[:].unsqueeze(2).broadcast_to((128, NBLK, 3, 3)),
            )
            zz = mm9[:, :, 2, 2]          # zc^2
            zxy = mm9[:, :, 2, 0:2]       # (zc xm, zc ym)
            xxyy = mm9[:].rearrange("p b i j -> p b (i j)")[:, :, 0:5:4]  # (xm^2, ym^2)
            xy2 = mm9[:, :, 0, 1]         # xm ym
            zz_b2 = zz.unsqueeze(2).broadcast_to((128, NBLK, 2))
            c22 = covf[:, :, 8:9]
            c22_b2 = c22.broadcast_to((128, NBLK, 2))

            # (q00, q11) pairwise:
            # q00 = zz c00 - zx (c02+c20) + xx c22 ; q11 = zz c11 - zy (c12+c21) + yy c22
            diag2 = covf[:, :, 0:5:4]     # (c00, c11)
            s12 = t(NBLK, 2)
            add(s12[:], cov[:, :, 0:2, 2], covf[:, :, 6:8])  # (c02+c20, c12+c21)
            qall = t(NBLK, 3)             # (q00, q11, q01)
            w0, w1 = t(NBLK, 2), t(NBLK, 2)
            mul(w0[:], zz_b2, diag2)
            mul(w1[:], zxy, s12[:])
            sub(w0[:], w0[:], w1[:])
            mul(w1[:], xxyy, c22_b2)
            add(qall[:, :, 0:2], w0[:], w1[:])

            # q01 = zz c01 - zy c02 - zx c21 + xm ym c22
            v0, v1 = t(NBLK), t(NBLK)
            mul(v0[:], zz, covf[:, :, 1])
            mul(v1[:], mm9[:, :, 2, 1], covf[:, :, 2])
            sub(v0[:], v0[:], v1[:])
            mul(v1[:], mm9[:, :, 2, 0], covf[:, :, 7])
            sub(v0[:], v0[:], v1[:])
            mul(v1[:], xy2, c22[:, :, 0])
            add(qall[:, :, 2], v0[:], v1[:])

            # wx = zc xm + (CX/FX) zz ; wy = zc ym + (CY/FY) zz  (both orders)
            wxy, wyx = t(NBLK, 2), t(NBLK, 2)
            stt(wxy[:, :, 0], zz, CX / FX, mm9[:, :, 2, 0], OP.mult, OP.add)
            stt(wxy[:, :, 1], zz, CY / FY, mm9[:, :, 2, 1], OP.mult, OP.add)
            nc.vector.tensor_copy(wyx[:, :, 0], wxy[:, :, 1])
            nc.vector.tensor_copy(wyx[:, :, 1], wxy[:, :, 0])

            # D = q00 q11 - q01^2 ; rD = 1/D ; uu = zz^2 ; ur = zz rD
            D, rD, uu, ur = t(NBLK), t(NBLK), t(NBLK), t(NBLK)
            mul(D[:], qall[:, :, 0], qall[:, :, 1])
            mul(v1[:], qall[:, :, 2], qall[:, :, 2])
            sub(D[:], D[:], v1[:])
            nc.vector.reciprocal(rD[:], D[:])
            mul(uu[:], zz, zz)
            mul(ur[:], zz, rD[:])

            coefs = t(NBLK, 6)
            # k=0..2: (q00, q11, q01) * uu * sc3 * rD
            w3 = t(NBLK, 3)
            mul(w3[:], qall[:], uu[:].unsqueeze(2).broadcast_to((128, NBLK, 3)))
            mul(w3[:], w3[:], sc3.unsqueeze(1).broadcast_to((128, NBLK, 3)))
            mul(
                coefs[:, :, 0:3], w3[:],
                rD[:].unsqueeze(2).broadcast_to((128, NBLK, 3)),
            )
            # k=3,4: (a_y, a_x) = (e1, e2) * ur * sc2 with
            #   e1 = q00 wy - q01 wx, e2 = q11 wx - q01 wy
            mul(w0[:], qall[:, :, 0:2], wyx[:])
            mul(
                w1[:],
                qall[:, :, 2:3].broadcast_to((128, NBLK, 2)), wxy[:],
            )
            sub(w0[:], w0[:], w1[:])
            # k=5: a_0 = (wy e1 + wx e2) rD  (regrouped quadratic form)
            ww = t(NBLK, 2)
            mul(ww[:], w0[:], wyx[:])
            mul(w0[:], w0[:], ur[:].unsqueeze(2).broadcast_to((128, NBLK, 2)))
            mul(coefs[:, :, 3:5], w0[:], sc2.unsqueeze(1).broadcast_to((128, NBLK, 2)))
            add(v1[:], ww[:, :, 0], ww[:, :, 1])
            mul(coefs[:, :, 5], v1[:], rD[:])

            # transpose [128, K] -> [K, 128] via PE, one per camera block
            identity = cpool.tile([128, 128], f32)
            masks.make_identity(nc, identity[:])
            coefTs = []
            for blk in range(NBLK):
                if mm_mode == "hilo":
                    # hi/lo bf16 split: rows = [hi, hi, lo] pairing with
                    # basis rows [hi, lo, hi];  dropped lo*lo ~ 2^-18 rel
                    chb = cpool.tile([128, 6], bf16, name=f"chb{blk}")
                    nc.vector.tensor_copy(chb[:], coefs[:, blk, :])
                    c18 = cpool.tile([128, 18], f32, name=f"c18_{blk}")
                    nc.vector.tensor_copy(c18[:, 0:6], chb[:])
                    nc.vector.tensor_copy(c18[:, 6:12], c18[:, 0:6])
                    nc.vector.tensor_sub(
                        c18[:, 12:18], coefs[:, blk, :], c18[:, 0:6]
                    )
                    src_ap = c18[:]
                else:
                    src_ap = coefs[:, blk, :]
                ptT = ptpool.tile([KROWS, 128], f32, name=f"ptT{blk}", bufs=1)
                nc.tensor.transpose(ptT[:], src_ap, identity[:])
                cT = cpool.tile([KROWS, 128], mmdt, name=f"coefT{blk}")
                nc.vector.tensor_copy(cT[:], ptT[:])
                coefTs.append(cT)

            # ---------------- main matmul + store loop -------------------
            # chunks per output DMA; first groups small so the store
            # stream starts as soon as the first matmul lands
            groups0 = [1, 1, 2] + [4] * ((NCHUNK - 4) // 4)
            groups1 = [4] * (NCHUNK // 4)
            gi = 0
            for blk in range(NBLK):
                lhsT = coefTs[blk][:]
                n = 0
                for grp in groups0 if blk == 0 else groups1:
                    n0 = n
                    ot = opool.tile([128, 4 * 512], f32, name="ot", tag="ot")
                    for q in range(grp):
                        pt = ppool.tile([128, 512], f32, name="pt", tag="pt")
                        nc.tensor.matmul(
                            pt[:],
                            lhsT,
                            basis[:, n * 512 : (n + 1) * 512],
                            start=True,
                            stop=True,
                        )
                        eng = nc.vector if q % 2 == 0 else nc.scalar
                        if eng is nc.vector:
                            eng.tensor_copy(ot[:, q * 512 : (q + 1) * 512], pt[:])
                        else:
                            eng.copy(ot[:, q * 512 : (q + 1) * 512], pt[:])
                        n += 1
                    deng = nc.sync if gi % 2 == 0 else nc.scalar
                    gi += 1
                    deng.dma_start(
                        out.ap()[blk, :, n0 * 512 : n * 512],
                        ot[:, 0 : grp * 512],
                    )

    nc.compile()
    return nc


def _get_nc(mm_mode=None):
    mm_mode = mm_mode or MM_MODE
    if mm_mode not in _cached:
        _cached[mm_mode] = _build(mm_mode)
    return _cached[mm_mode]


def make_in_maps(means_hom_tmp, x, cov_world, tile_coord):
    xf = np.ascontiguousarray(x, dtype=np.float32).reshape(B, 16)
    consts = np.concatenate(
        [
            np.asarray(means_hom_tmp, dtype=np.float32).reshape(4),
            np.asarray(cov_world, dtype=np.float32).reshape(9),
            np.array(
                [
                    1.0 / (FY * FY), 1.0 / (FX * FX), -2.0 / (FX * FY),
                    -2.0 / FY, -2.0 / FX,
                ],
                dtype=np.float32,
            ),
        ]
    )  # [18]
    tc2 = np.asarray(tile_coord, dtype=np.float32).reshape(P, 2)
    tx, ty = tc2[:, 0], tc2[:, 1]
    bss = np.stack(
        [ty * ty, tx * tx, tx * ty, ty, tx, np.ones(P, np.float32)]
    ).astype(np.float32)  # [6, P], basis row order matches the kernel
    if MM_MODE == "hilo":
        import ml_dtypes

        bh = bss.astype(ml_dtypes.bfloat16)
        bl = (bss - bh.astype(np.float32)).astype(ml_dtypes.bfloat16)
        bss = np.concatenate([bh, bl, bh], axis=0)  # [18, P] bf16
    maps = []
    for i in range(NCORES):
        xloc = xf[i * BLOC : (i + 1) * BLOC].reshape(NBLK, 128, 16)
        xcc = np.zeros((128, 56), dtype=np.float32)
        xcc[:, 0:32] = xloc.transpose(1, 0, 2).reshape(128, 32)
        xcc[:, 32:50] = consts[None, :]
        maps.append({"xc": xcc, "bs": bss})
    return maps


def kernel(means_hom_tmp, x, cov_world, opacities_rast=None, tile_coord=None):
    from concourse.bass_utils import run_bass_kernel_spmd

    nc = _get_nc()
    in_maps = make_in_maps(means_hom_tmp, x, cov_world, tile_coord)
    res = run_bass_kernel_spmd(nc, in_maps, list(range(NCORES)))
    outs = [res.results[i]["out"].reshape(BLOC, P) for i in range(NCORES)]
    return np.concatenate(outs, axis=0)


# revision 26
# speedup vs baseline: 1.1470x; 1.1470x over previous
"""Trainium2 Bass kernel for nn_AlphaModel (3DGS EWA conic rasterization term).

Math: the reference output inside[b, p] is a quadratic polynomial in the pixel
coordinates (tx, ty) with per-camera coefficients:

    inside[b,p] = a_yy[b]*ty^2 + a_xx[b]*tx^2 + a_xy[b]*tx*ty
                + a_y[b]*ty + a_x[b]*tx + a_0[b]

so the [B, P] output is a rank-6 contraction  coef[B,6] @ basis[6,P].
Per-camera coefficients are derived on-device from x (the sharded input);
the basis rows are polynomial features of tile_coord (a tiny replicated
constant) prepared host-side during sharding.  Data-parallel over B across
8 cores; no cross-device communication.

Derivation (with q00/q11/q01 the FX/FY-factored 2D-covariance entries,
wx = vx/FX, wy = vy/FY, D = q00*q11 - q01^2, u = zc^2):
    a_yy = q00 u^2/(FY^2 D)        a_xx = q11 u^2/(FX^2 D)
    a_xy = -2 q01 u^2/(FX FY D)
    a_y  = -2u (q00 wy - q01 wx)/(FY D)
    a_x  = -2u (q11 wx - q01 wy)/(FX D)
    a_0  = (q11 wx^2 + q00 wy^2 - 2 q01 wx wy)/D
Basis row order: ty^2, tx^2, tx*ty, ty, tx, 1.
"""

import os

import numpy as np

B = 2048
P = 16384
NCORES = 8
BLOC = B // NCORES          # 256 cameras per core
NBLK = BLOC // 128          # 2 partition blocks per core
NCHUNK = P // 512           # 32 pixel chunks of 512
FX = 2343.0242837919386
FY = 2343.0242837919386
CX = 2560 / 2.0
CY = 1440 / 2.0

# matmul dtype strategy:
#   "f32"  - exact fp32 PE matmul (4 cyc/row, slow)
#   "f32r" - single-pass reduced-precision fp32 (1 cyc/row, ~7e-4 rel err)
#   "hilo" - bf16 hi/lo split, K=18 (1 cyc/row, ~1e-5 rel err)
MM_MODE = os.environ.get("ALPHA_MM_MODE", "hilo")

_cached = {}


def _build(mm_mode: str):
    import concourse.bacc as bacc
    import concourse.mybir as mybir
    import concourse.tile as tile
    from concourse import masks

    f32 = mybir.dt.float32
    bf16 = mybir.dt.bfloat16
    mmdt = {"f32": f32, "f32r": mybir.dt.float32r, "hilo": bf16}[mm_mode]
    KROWS = 18 if mm_mode == "hilo" else 6
    AX = mybir.AxisListType
    OP = mybir.AluOpType

    nc = bacc.Bacc("TRN2", target_bir_lowering=False, debug=False)

    xc = nc.dram_tensor("xc", [128, 56], f32, kind="ExternalInput")
    bs = nc.dram_tensor("bs", [KROWS, P], mmdt, kind="ExternalInput")
    out = nc.dram_tensor("out", [NBLK, 128, P], f32, kind="ExternalOutput")

    with tile.TileContext(nc) as tc:
        with (
            tc.tile_pool(name="const", bufs=1) as cpool,
            tc.tile_pool(name="obuf", bufs=4) as opool,
            tc.tile_pool(name="psum", bufs=6, space="PSUM") as ppool,
            tc.tile_pool(name="psumT", bufs=1, space="PSUM") as ptpool,
        ):
            # ------- input loads: coef-chain inputs first, own rings -----
            xcb = cpool.tile([128, 56], f32)
            nc.sync.dma_start(xcb[:], xc.ap())
            basis = cpool.tile([KROWS, P], mmdt)
            BW = P // 8
            for k in range(8):
                deng = nc.scalar if k % 2 == 0 else nc.sync
                deng.dma_start(
                    basis[:, k * BW : (k + 1) * BW],
                    bs.ap()[:, k * BW : (k + 1) * BW],
                )

            mb = xcb[:, 32:36]
            ab = xcb[:, 36:45]
            sc3 = xcb[:, 45:48]   # 1/FY^2, 1/FX^2, -2/(FX FY)
            sc2 = xcb[:, 48:50]   # -2/FY, -2/FX

            # ---------------- per-camera coefficients --------------------
            _tn = [0]

            def t(*shape):
                _tn[0] += 1
                return cpool.tile([128, *shape], f32, name=f"sc{_tn[0]}")

            mul, add, sub = (
                nc.vector.tensor_mul,
                nc.vector.tensor_add,
                nc.vector.tensor_sub,
            )

            def stt(o, i0, s, i1, op0, op1):
                nc.vector.scalar_tensor_tensor(o, i0, float(s), i1, op0=op0, op1=op1)

            # means_cam[b, i] = sum_j x[b, i, j] * m[j]   (both blocks at once)
            xv = xcb[:, 0:32].rearrange("p (b i j) -> p b i j", b=NBLK, i=4)[
                :, :, 0:3, :
            ]  # [128, b, i, j=4]
            tmp_mc = t(NBLK, 3, 4)
            mul(
                tmp_mc[:], xv,
                mb.unsqueeze(1).unsqueeze(1).broadcast_to((128, NBLK, 3, 4)),
            )
            mc = t(NBLK, 3)
            nc.vector.reduce_sum(mc[:], tmp_mc[:], axis=AX.X)

            # cov_cam = R A R^T per block (ISA limit: 3 free dims -> per blk)
            cov = t(NBLK, 3, 3)
            av = ab.rearrange("p (j l) -> p l j", l=3)  # [128, l, j]
            for blk in range(NBLK):
                rvb = xcb[:, blk * 16 : blk * 16 + 12].rearrange(
                    "p (i j) -> p i j", j=4
                )[:, :, 0:3]  # [128, i, j]
                tmpT = t(3, 3, 3)  # [128, i, l, j]
                mul(
                    tmpT[:],
                    rvb.unsqueeze(2).broadcast_to((128, 3, 3, 3)),
                    av.unsqueeze(1).broadcast_to((128, 3, 3, 3)),
                )
                TT = t(3, 3)  # [128, i, l]
                nc.vector.reduce_sum(TT[:], tmpT[:], axis=AX.X)
                tmpC = t(3, 3, 3)  # [128, i, k, l]
                mul(
                    tmpC[:],
                    TT[:].unsqueeze(2).broadcast_to((128, 3, 3, 3)),
                    rvb.unsqueeze(1).broadcast_to((128, 3, 3, 3)),
                )
                nc.vector.reduce_sum(cov[:, blk, :, :], tmpC[:], axis=AX.X)

            covf = cov[:].rearrange("p b i k -> p b (i k)")

            # mm9[b,i,j] = mc_i * mc_j  (outer product of means)
            mm9 = t(NBLK, 3, 3)
            mul(
                mm9[:],
                mc[:].unsqueeze(3).broadcast_to((128, NBLK, 3, 3)),
                mc[:].unsqueeze(2).broadcast_to((128, NBLK, 3, 3)),
            )
            zz = mm9[:, :, 2, 2]          # zc^2
            zxy = mm9[:, :, 2, 0:2]       # (zc xm, zc ym)
            xxyy = mm9[:].rearrange("p b i j -> p b (i j)")[:, :, 0:5:4]  # (xm^2, ym^2)
            xy2 = mm9[:, :, 0, 1]         # xm ym
            zz_b2 = zz.unsqueeze(2).broadcast_to((128, NBLK, 2))
            c22 = covf[:, :, 8:9]
            c22_b2 = c22.broadcast_to((128, NBLK, 2))

            # (q00, q11) pairwise:
            # q00 = zz c00 - zx (c02+c20) + xx c22 ; q11 = zz c11 - zy (c12+c21) + yy c22
            diag2 = covf[:, :, 0:5:4]     # (c00, c11)
            s12 = t(NBLK, 2)
            add(s12[:], cov[:, :, 0:2, 2], covf[:, :, 6:8])  # (c02+c20, c12+c21)
            qall = t(NBLK, 3)             # (q00, q11, q01)
            w0, w1 = t(NBLK, 2), t(NBLK, 2)
            mul(w0[:], zz_b2, diag2)
            mul(w1[:], zxy, s12[:])
            sub(w0[:], w0[:], w1[:])
            mul(w1[:], xxyy, c22_b2)
            add(qall[:, :, 0:2], w0[:], w1[:])

            # q01 = zz c01 - zy c02 - zx c21 + xm ym c22
            v0, v1 = t(NBLK), t(NBLK)
            mul(v0[:], zz, covf[:, :, 1])
            mul(v1[:], mm9[:, :, 2, 1], covf[:, :, 2])
            sub(v0[:], v0[:], v1[:])
            mul(v1[:], mm9[:, :, 2, 0], covf[:, :, 7])
            sub(v0[:], v0[:], v1[:])
            mul(v1[:], xy2, c22[:, :, 0])
            add(qall[:, :, 2], v0[:], v1[:])

            # wx = zc xm + (CX/FX) zz ; wy = zc ym + (CY/FY) zz  (both orders)
            wxy, wyx = t(NBLK, 2), t(NBLK, 2)
            stt(wxy[:, :, 0], zz, CX / FX, mm9[:, :, 2, 0], OP.mult, OP.add)
            stt(wxy[:, :, 1], zz, CY / FY, mm9[:, :, 2, 1], OP.mult, OP.add)
            nc.vector.tensor_copy(wyx[:, :, 0], wxy[:, :, 1])
            nc.vector.tensor_copy(wyx[:, :, 1], wxy[:, :, 0])

            # D = q00 q11 - q01^2 ; rD = 1/D ; uu = zz^2 ; ur = zz rD
            D, rD, uu, ur = t(NBLK), t(NBLK), t(NBLK), t(NBLK)
            mul(D[:], qall[:, :, 0], qall[:, :, 1])
            mul(v1[:], qall[:, :, 2], qall[:, :, 2])
            sub(D[:], D[:], v1[:])
            nc.vector.reciprocal(rD[:], D[:])
            mul(uu[:], zz, zz)
            mul(ur[:], zz, rD[:])

            coefs = t(NBLK, 6)
            # k=0..2: (q00, q11, q01) * uu * sc3 * rD
            w3 = t(NBLK, 3)
            mul(w3[:], qall[:], uu[:].unsqueeze(2).broadcast_to((128, NBLK, 3)))
            mul(w3[:], w3[:], sc3.unsqueeze(1).broadcast_to((128, NBLK, 3)))
            mul(
                coefs[:, :, 0:3], w3[:],
                rD[:].unsqueeze(2).broadcast_to((128, NBLK, 3)),
            )
            # k=3,4: (a_y, a_x) = (e1, e2) * ur * sc2 with
            #   e1 = q00 wy - q01 wx, e2 = q11 wx - q01 wy
            mul(w0[:], qall[:, :, 0:2], wyx[:])
            mul(
                w1[:],
                qall[:, :, 2:3].broadcast_to((128, NBLK, 2)), wxy[:],
            )
            sub(w0[:], w0[:], w1[:])
            # k=5: a_0 = (wy e1 + wx e2) rD  (regrouped quadratic form)
            ww = t(NBLK, 2)
            mul(ww[:], w0[:], wyx[:])
            mul(w0[:], w0[:], ur[:].unsqueeze(2).broadcast_to((128, NBLK, 2)))
            mul(coefs[:, :, 3:5], w0[:], sc2.unsqueeze(1).broadcast_to((128, NBLK, 2)))
            add(v1[:], ww[:, :, 0], ww[:, :, 1])
            mul(coefs[:, :, 5], v1[:], rD[:])

            # transpose [128, K] -> [K, 128] via PE, one per camera block
            identity = cpool.tile([128, 128], f32)
            masks.make_identity(nc, identity[:])
            coefTs = []
            for blk in range(NBLK):
                if mm_mode == "hilo":
                    # hi/lo bf16 split: rows = [hi, hi, lo] pairing with
                    # basis rows [hi, lo, hi];  dropped lo*lo ~ 2^-18 rel
                    chb = cpool.tile([128, 6], bf16, name=f"chb{blk}")
                    nc.vector.tensor_copy(chb[:], coefs[:, blk, :])
                    c18 = cpool.tile([128, 18], f32, name=f"c18_{blk}")
                    nc.vector.tensor_copy(c18[:, 0:6], chb[:])
                    nc.vector.tensor_copy(c18[:, 6:12], c18[:, 0:6])
                    nc.vector.tensor_sub(
                        c18[:, 12:18], coefs[:, blk, :], c18[:, 0:6]
                    )
                    src_ap = c18[:]
                else:
                    src_ap = coefs[:, blk, :]
                ptT = ptpool.tile([KROWS, 128], f32, name=f"ptT{blk}", bufs=1)
                nc.tensor.transpose(ptT[:], src_ap, identity[:])
                cT = cpool.tile([KROWS, 128], mmdt, name=f"coefT{blk}")
                nc.vector.tensor_copy(cT[:], ptT[:])
                coefTs.append(cT)

            # ---------------- main matmul + store loop -------------------
            # chunks per output DMA; first groups small so the store
            # stream starts as soon as the first matmul lands
            groups0 = [1, 1, 2] + [4] * ((NCHUNK - 4) // 4)
            groups1 = [4] * (NCHUNK // 4)
            gi = 0
            for blk in range(NBLK):
                lhsT = coefTs[blk][:]
                n = 0
                for grp in groups0 if blk == 0 else groups1:
                    n0 = n
                    ot = opool.tile([128, 4 * 512], f32, name="ot", tag="ot")
                    for q in range(grp):
                        pt = ppool.tile([128, 512], f32, name="pt", tag="pt")
                        nc.tensor.matmul(
                            pt[:],
                            lhsT,
                            basis[:, n * 512 : (n + 1) * 512],
                            start=True,
                            stop=True,
                        )
                        eng = nc.vector if q % 2 == 0 else nc.scalar
                        if eng is nc.vector:
                            eng.tensor_copy(ot[:, q * 512 : (q + 1) * 512], pt[:])
                        else:
                            eng.copy(ot[:, q * 512 : (q + 1) * 512], pt[:])
                        n += 1
                    deng = nc.sync if gi % 2 == 0 else nc.scalar
                    gi += 1
                    deng.dma_start(
                        out.ap()[blk, :, n0 * 512 : n * 512],
                        ot[:, 0 : grp * 512],
                    )

    nc.compile()
    return nc


def _get_nc(mm_mode=None):
    mm_mode = mm_mode or MM_MODE
    if mm_mode not in _cached:
        _cached[mm_mode] = _build(mm_mode)
    return _cached[mm_mode]


def make_in_maps(means_hom_tmp, x, cov_world, tile_coord, mm_mode=None):
    mm_mode = mm_mode or MM_MODE
    xf = np.ascontiguousarray(x, dtype=np.float32).reshape(B, 16)
    consts = np.concatenate(
        [
            np.asarray(means_hom_tmp, dtype=np.float32).reshape(4),
            np.asarray(cov_world, dtype=np.float32).reshape(9),
            np.array(
                [
                    1.0 / (FY * FY), 1.0 / (FX * FX), -2.0 / (FX * FY),
                    -2.0 / FY, -2.0 / FX,
                ],
                dtype=np.float32,
            ),
        ]
    )  # [18]
    tc2 = np.asarray(tile_coord, dtype=np.float32).reshape(P, 2)
    tx, ty = tc2[:, 0], tc2[:, 1]
    bss = np.stack(
        [ty * ty, tx * tx, tx * ty, ty, tx, np.ones(P, np.float32)]
    ).astype(np.float32)  # [6, P], basis row order matches the kernel
    if mm_mode == "hilo":
        import ml_dtypes

        bh = bss.astype(ml_dtypes.bfloat16)
        bl = (bss - bh.astype(np.float32)).astype(ml_dtypes.bfloat16)
        bss = np.concatenate([bh, bl, bh], axis=0)  # [18, P] bf16
    maps = []
    for i in range(NCORES):
        xloc = xf[i * BLOC : (i + 1) * BLOC].reshape(NBLK, 128, 16)
        xcc = np.zeros((128, 56), dtype=np.float32)
        xcc[:, 0:32] = xloc.transpose(1, 0, 2).reshape(128, 32)
        xcc[:, 32:50] = consts[None, :]
        maps.append({"xc": xcc, "bs": bss})
    return maps


def _ensure_axon_hooks():
    """bass_utils' trace path imports antenv.axon_hooks, which some agent
    images lack; synthesize it (mirroring trn_agent_boot) so tracing
    degrades gracefully instead of crashing."""
    try:
        import antenv.axon_hooks  # noqa: F401
        return
    except ImportError:
        pass
    import contextlib
    import ctypes
    import sys
    import types

    mod = types.ModuleType("antenv.axon_hooks")
    mod._HOOK = None

    def set_axon_ntff_profile_hook(hook):
        mod._HOOK = hook

    def get_axon_ntff_profile_hook():
        if mod._HOOK is not None:
            return mod._HOOK
        so_path = "/opt/axon/libaxon_pjrt.so"
        if not os.path.exists(so_path):
            return None
        try:
            lib = ctypes.CDLL(so_path)
        except OSError:
            return None
        if not hasattr(lib, "axon_start_nrt_profile"):
            return None
        lib.axon_start_nrt_profile.argtypes = [
            ctypes.POINTER(ctypes.c_int64), ctypes.c_size_t,
        ]
        lib.axon_start_nrt_profile.restype = ctypes.c_int64
        lib.axon_stop_nrt_profile.argtypes = [ctypes.c_char_p]
        lib.axon_stop_nrt_profile.restype = ctypes.c_int64

        @contextlib.contextmanager
        def _hook(output_dir, device_ids):
            import jax

            jax.devices()
            if device_ids:
                ids = (ctypes.c_int64 * len(device_ids))(*device_ids)
                rc = lib.axon_start_nrt_profile(ids, len(device_ids))
            else:
                rc = lib.axon_start_nrt_profile(None, 0)
            if rc != 0:
                raise RuntimeError(f"axon_start_nrt_profile rc={rc}")
            try:
                yield
            finally:
                lib.axon_stop_nrt_profile(str(output_dir).encode())

        return _hook

    mod.set_axon_ntff_profile_hook = set_axon_ntff_profile_hook
    mod.get_axon_ntff_profile_hook = get_axon_ntff_profile_hook
    sys.modules["antenv.axon_hooks"] = mod
    try:
        import antenv

        antenv.axon_hooks = mod
    except ImportError:
        pass


def kernel(means_hom_tmp, x, cov_world, opacities_rast=None, tile_coord=None):
    _ensure_axon_hooks()
    from concourse.bass_utils import run_bass_kernel_spmd

    nc = _get_nc()
    in_maps = make_in_maps(means_hom_tmp, x, cov_world, tile_coord)
    res = run_bass_kernel_spmd(nc, in_maps, list(range(NCORES)))
    outs = [res.results[i]["out"].reshape(BLOC, P) for i in range(NCORES)]
    return np.concatenate(outs, axis=0)
